# revision 1
# baseline (speedup 1.0000x reference)
"""Cox proportional-hazards loss (CoxNNet) on 8 Trainium2 NeuronCores.

loss = -mean((theta - log(risk_sum)) * events)
risk_sum[i] = sum_j [d_j >= d_i] * exp(theta_j)        (N = 16384)

Sharding: rows i of the [N, N] risk-set reduction are split across 8 cores
(2048 rows each). Each core receives ONLY its shard — one packed [6144] f32
input (d_shard | theta_shard | events_shard, 24 KiB) — and reconstructs the
full d/theta vectors on-device with a single HBM-HBM AllGather over the 8
cores (16 KiB per core on NeuronLink vs. ~1 MiB of host-replicated input
over the axon tunnel at ~10 ms/MiB).

Per-core compute:
  - d, theta land in a [128, 128] chunk layout (partition = fast index),
  - w = exp(theta) on the scalar engine,
  - the core's 2048 d_i values are broadcast across 128 partitions (K=1
    matmul),
  - for each of 128 j-chunks: DVE (cols 0-1535) and GPSIMD (cols 1536-2047)
    build the [128-j x 2048-i] exact is_le 0/1 mask; 4 K=1 float32r matmuls
    (full-rate PE) accumulate risk_sum into 4 [1, 512] PSUM banks,
  - epilogue: risk -> ln -> (theta_i - ln) * e_i -> free-dim reduce
    -> one f32 partial per core.
Host combines: loss = -(sum of partials) / N.

Launch path: the stock run_bass_kernel_spmd re-creates and re-jits a fresh
shard_map closure on EVERY call (~180 ms of retrace + lowering per launch).
This module builds the jitted executable ONCE and reuses it, so a
steady-state launch is a single axon round-trip (transfers + execute +
fetch pipeline into one sync).
"""

import numpy as np

import concourse.bass as bass
import concourse.bacc as bacc
import concourse.mybir as mybir
from concourse.tile import TileContext

N = 16384
P = 128
NCH = N // P            # 128 j-chunks per core (all j)
NCORES = 8
NI = N // NCORES        # 2048 i-rows per core
FT = 512                # fp32 moving-operand max / one PSUM bank
NF = NI // FT           # 4 PSUM accumulators

# Mask generation engine split: DVE tensor_scalar(is_le) takes cols
# [0, DVE_COLS); GPSIMD takes the rest with the same exact is_le (idle
# after the prelude collective; ~0.92 ns/col at 0.6x roofline + 95 ns
# launch per instruction). ACT Sign lost its seat: ~1.4 ns/col + ~1.3 µs
# fixed beats neither, and is_le everywhere needs no tie fixup. 1536/512
# puts DVE (~108 µs) just under the PE stream (~113 µs).
DVE_COLS = 1536
F32 = mybir.dt.float32
# Main-loop matmuls stream f32 data as float32r: full-rate PE (1 cycle/row
# at moving free dim >= 512 vs 4 for plain fp32). The 0/1 masks are exact
# under any mantissa truncation; only w picks up <=1e-3 per-term error on
# hardware (fp32r is the one dtype where CoreSim and HW numerics may
# diverge), far inside the 2e-2 gate. The d-broadcast matmuls stay plain
# fp32 so bc_di is bit-exact for the is_le comparisons.
F32R = mybir.dt.float32r


def _build(dve_cols: int = DVE_COLS):
    gp_cols = NI - dve_cols
    # disable_frame_to_traceback keeps python file/line out of the BIR, so
    # the compiled artifact (and its cache key) is independent of the
    # directory kernel.py is imported from — a fresh checkout reuses the
    # warmed compile instead of paying the ~2 min neuronx-cc run.
    nc = bacc.Bacc(num_devices=NCORES, disable_frame_to_traceback=True)
    packed = nc.declare_dram_parameter("packed", [3 * NI], F32, isOutput=False)
    out = nc.declare_dram_parameter("partial", [1, 1], F32, isOutput=True)

    with TileContext(nc) as tc:
        with (
            tc.tile_pool(name="dram", bufs=1, space="DRAM") as dpool,
            tc.tile_pool(name="const", bufs=1) as cpool,
            tc.tile_pool(name="mask", bufs=4) as mpool,
            tc.tile_pool(name="acc", bufs=1, space="PSUM") as ppool,
            tc.tile_pool(name="bc", bufs=2, space="PSUM") as bcpool,
        ):
            # collective bounce buffers (collectives can't touch I/O tensors)
            dt_in = dpool.tile([1, 2 * NI], F32)     # my d_shard | theta_shard
            dt_all = dpool.tile([1, 2 * N], F32)     # 8x (d_g | theta_g)

            sb_d = cpool.tile([P, NCH], F32)    # d[p*128 + c] at [p, c]
            sb_th = cpool.tile([P, NCH], F32)
            w_act = cpool.tile([P, NCH], F32)   # exp(theta), ACT-written
            # f32r: BIR requires fp32r matmul operands to be written
            # f32r-rounded by their producer, so the tile is typed f32r and
            # the DVE copy does the rounding.
            w_sb = cpool.tile([P, NCH], F32R)   # DVE copy (single-engine deps for PE)
            ones_row = cpool.tile([1, P], F32)  # bcast lhsT [K=1, M=128]
            row_di = cpool.tile([1, NI], F32)
            row_di2 = cpool.tile([1, NI], F32)  # DVE copy of row_di
            row_thi = cpool.tile([1, NI], F32)
            row_ei = cpool.tile([1, NI], F32)
            bc_di = cpool.tile([P, NI], F32)
            risk_row = cpool.tile([1, NI], F32)
            ln_row = cpool.tile([1, NI], F32)
            diff_row = cpool.tile([1, NI], F32)
            prod_row = cpool.tile([1, NI], F32)
            part_sb = cpool.tile([1, 1], F32)

            # ---- gather the full d/theta from the 8 shards ----
            nc.gpsimd.dma_start(
                out=dt_in[:, :],
                in_=packed[:2 * NI].rearrange("(o n) -> o n", o=1),
            )
            nc.gpsimd.collective_compute(
                "AllGather",
                mybir.AluOpType.bypass,
                replica_groups=[list(range(NCORES))],
                ins=[dt_in.opt()],
                outs=[dt_all.opt()],
            )

            # ---- loads ----
            # dt_all[g*4096 : g*4096+2048] = d for global rows [2048g, 2048(g+1))
            # → sb_d[16g:16(g+1), :] (global j = p*128 + c sits at [p, c]);
            # next 2048 are the matching theta block. Per-g DMAs are forced:
            # the interleaved d|theta gather layout makes the (g, p') partition
            # strides non-mergeable (4096 vs 16*128) into one uniform-stride AP.
            for g in range(NCORES):
                o = g * 2 * NI
                nc.sync.dma_start(
                    out=sb_d[16 * g:16 * (g + 1), :],
                    in_=dt_all[0, o:o + NI].rearrange("(p c) -> p c", p=16),
                )
                nc.sync.dma_start(
                    out=sb_th[16 * g:16 * (g + 1), :],
                    in_=dt_all[0, o + NI:o + 2 * NI].rearrange("(p c) -> p c", p=16),
                )
            nc.sync.dma_start(out=row_di[:, :],
                              in_=packed[:NI].rearrange("(o n) -> o n", o=1))
            nc.sync.dma_start(out=row_thi[:, :],
                              in_=packed[NI:2 * NI].rearrange("(o n) -> o n", o=1))
            nc.sync.dma_start(out=row_ei[:, :],
                              in_=packed[2 * NI:].rearrange("(o n) -> o n", o=1))

            # ---- prep ----
            # PE allows only ONE sync wait per Matmult: funnel every matmul
            # input through the vector engine so PE waits on a single DVE sem.
            nc.scalar.activation(w_act[:, :], sb_th[:, :], mybir.ActivationFunctionType.Exp)
            nc.vector.tensor_copy(w_sb[:, :], w_act[:, :])
            nc.vector.memset(ones_row[:, :], 1.0)
            nc.vector.tensor_copy(row_di2[:, :], row_di[:, :])
            for t in range(NF):
                bc_ps = bcpool.tile([P, FT], F32, tag="bc")
                nc.tensor.matmul(
                    bc_ps[:, :], lhsT=ones_row[:, :],
                    rhs=row_di2[:, t * FT:(t + 1) * FT], start=True, stop=True,
                )
                nc.vector.tensor_copy(bc_di[:, t * FT:(t + 1) * FT], bc_ps[:, :])

            # ---- main loop: mask gen + masked reduce ----
            risk_ps = [ppool.tile([1, FT], F32, name=f"risk{t}") for t in range(NF)]
            assert dve_cols % FT == 0, "engine split must align to matmul tiles"
            for c in range(NCH):
                # separate tiles per producing engine — a shared tile would
                # WAW-serialize DVE behind GPSIMD in the Tile dep tracker
                mask_d = None
                mask_g = None
                if dve_cols > 0:
                    mask_d = mpool.tile([P, dve_cols], F32R, tag="mask_d",
                                        name=f"mask_d{c}")
                if gp_cols > 0:
                    mask_g = mpool.tile([P, gp_cols], F32R, tag="mask_g",
                                        name=f"mask_g{c}")
                if mask_d is not None:
                    nc.vector.tensor_scalar(
                        mask_d[:, :], bc_di[:, :dve_cols],
                        sb_d[:, c:c + 1], None, mybir.AluOpType.is_le,
                    )
                if mask_g is not None:
                    nc.gpsimd.tensor_scalar(
                        mask_g[:, :], bc_di[:, dve_cols:],
                        sb_d[:, c:c + 1], None, mybir.AluOpType.is_le,
                    )
                for t in range(NF):
                    lo = t * FT
                    if lo < dve_cols:
                        rhs = mask_d[:, lo:lo + FT]
                    else:
                        rhs = mask_g[:, lo - dve_cols:lo - dve_cols + FT]
                    nc.tensor.matmul(
                        risk_ps[t][:, :], lhsT=w_sb[:, c:c + 1],
                        rhs=rhs,
                        start=(c == 0), stop=(c == NCH - 1),
                    )

            # ---- epilogue ----
            for t in range(NF):
                nc.vector.tensor_copy(risk_row[:, t * FT:(t + 1) * FT],
                                      risk_ps[t][:, :])

            # (tensor_tensor_reduce crashes at runtime on this stack — use
            # separate mul + reduce_sum instead)
            nc.scalar.activation(ln_row[:, :], risk_row[:, :],
                                 mybir.ActivationFunctionType.Ln)
            nc.vector.tensor_sub(diff_row[:, :], row_thi[:, :], ln_row[:, :])
            nc.vector.tensor_mul(prod_row[:, :], diff_row[:, :], row_ei[:, :])
            nc.vector.reduce_sum(part_sb[:, :], prod_row[:, :],
                                 axis=mybir.AxisListType.X)
            nc.sync.dma_start(out=out[:, :], in_=part_sb[:, :])

    nc.finalize()
    return nc


def _make_cached_runner(nc):
    """One-time: lower nc to a jitted shard_map executable and keep it.

    Mirrors bass2jax.run_bass_via_pjrt, but hoists everything reusable out
    of the per-call path — the stock helper rebuilds + re-jits a fresh
    closure per call, which costs ~180 ms of retrace/lowering per launch.
    """
    import jax
    from jax.experimental.shard_map import shard_map
    from jax.sharding import Mesh, PartitionSpec

    from concourse.bass2jax import (
        _bass_exec_p,
        install_neuronx_cc_hook,
        partition_id_tensor,
    )

    install_neuronx_cc_hook()

    # The serialized BIR is embedded verbatim in the HLO custom_call, so
    # every compile-cache key downstream inherits its byte content. The
    # ant_debug strings embed this file's absolute path, which would make
    # the cache key depend on the directory kernel.py is imported from
    # (fresh checkout -> ~2 min recompile). Scrub the path so the artifact
    # is byte-identical everywhere; debug info is otherwise untouched.
    import os
    _path = os.path.abspath(__file__).encode()
    _orig_to_json_bytes = nc.to_json_bytes

    def _scrubbed_to_json_bytes():
        return _orig_to_json_bytes().replace(_path, b"kernel.py")

    nc.to_json_bytes = _scrubbed_to_json_bytes

    partition_name = nc.partition_id_tensor.name if nc.partition_id_tensor else None

    in_names, out_names, out_avals, out_shapes = [], [], [], []
    for alloc in nc.m.functions[0].allocations:
        if not isinstance(alloc, mybir.MemoryLocationSet):
            continue
        name = alloc.memorylocations[0].name
        if alloc.kind == "ExternalInput":
            if name != partition_name:
                in_names.append(name)
        elif alloc.kind == "ExternalOutput":
            out_names.append(name)
            shape = tuple(alloc.tensor_shape)
            out_shapes.append(shape)
            out_avals.append(jax.core.ShapedArray(shape, mybir.dt.np(alloc.dtype)))
    assert in_names == ["packed"] and out_names == ["partial"]
    n_params = len(in_names)
    n_outs = len(out_avals)
    all_in_names = in_names + out_names
    if partition_name is not None:
        all_in_names.append(partition_name)
    donate = tuple(range(n_params, n_params + n_outs))

    def _body(*args):
        operands = list(args)
        if partition_name is not None:
            operands.append(partition_id_tensor())
        outs = _bass_exec_p.bind(
            *operands,
            out_avals=tuple(out_avals),
            in_names=tuple(all_in_names),
            out_names=tuple(out_names),
            lowering_input_output_aliases=(),
            sim_require_finite=True,
            sim_require_nnan=True,
            nc=nc,
        )
        return tuple(outs)

    devices = jax.devices()[:NCORES]
    assert len(devices) == NCORES, f"need {NCORES} cores, have {len(jax.devices())}"
    mesh = Mesh(np.asarray(devices), ("core",))
    in_specs = (PartitionSpec("core"),) * (n_params + n_outs)
    out_specs = (PartitionSpec("core"),) * n_outs
    sharded = jax.jit(
        shard_map(_body, mesh=mesh, in_specs=in_specs, out_specs=out_specs,
                  check_rep=False),
        donate_argnums=donate,
        keep_unused=True,
    )

    def run(packed_all):
        """packed_all: [NCORES * 3*NI] f32 → per-core partial sums [NCORES]."""
        zeros = np.zeros((NCORES * out_shapes[0][0], *out_shapes[0][1:]),
                         np.float32)
        out_arrs = sharded(packed_all, zeros)
        return np.asarray(out_arrs[0]).reshape(-1)

    return run


_RUNNER = None


def _get_runner():
    global _RUNNER
    if _RUNNER is None:
        _RUNNER = _make_cached_runner(_build())
    return _RUNNER


def _pack_inputs(hazard_pred, durations, events):
    theta = np.asarray(hazard_pred, dtype=np.float32).reshape(-1)
    d = np.asarray(durations, dtype=np.float32).reshape(-1)
    e = np.asarray(events, dtype=np.float32).reshape(-1)
    packed = np.empty((NCORES, 3 * NI), np.float32)
    packed[:, :NI] = d.reshape(NCORES, NI)
    packed[:, NI:2 * NI] = theta.reshape(NCORES, NI)
    packed[:, 2 * NI:] = e.reshape(NCORES, NI)
    return packed.reshape(-1)


def kernel(hazard_pred, durations, events):
    runner = _get_runner()
    partials = runner(_pack_inputs(hazard_pred, durations, events))
    loss = -(np.sum(partials.astype(np.float64)) / N)
    return np.asarray(loss, dtype=np.float32)


def run(hazard_pred, durations, events, trace=False, dve_cols=DVE_COLS, **kw):
    """test.py compatibility shim (trace/dve_cols accepted and ignored)."""
    return kernel(hazard_pred, durations, events), None



# revision 2
# speedup vs baseline: 314.1366x; 314.1366x over previous
"""Cox proportional-hazards loss (CoxNNet) on 8 Trainium2 NeuronCores.

loss = -mean((theta - log(risk_sum)) * events)
risk_sum[i] = sum_j [d_j >= d_i] * exp(theta_j)        (N = 16384)

Rows i of the [N, N] risk-set reduction are split across 8 cores (2048 rows
each); durations/theta are host-replicated to every core (the per-launch
wall is round-trip dominated, and skipping the on-device AllGather removes
~20us of collective latency from the kernel's critical path).

Per-core kernel (measured ~170us on HW vs ~1.1ms for the f32r/GPSIMD
baseline measured the same way):
  - d, theta land in a [128, 128] chunk layout (global j = 128p + c at
    [p, c]; per-partition-contiguous DMA),
  - bc_di = broadcast of the core's 2048 d_i values across partitions via
    K=1 f32r matmuls, copied out once in f32 and once in bf16,
  - per chunk c (128 j's): ONE DVE tensor_scalar is_le builds the
    [128, 2048] bf16 mask from the bf16 broadcast - bf16 in + bf16 out
    engages the DVE 4x perf mode (~0.5us/chunk; fp8 or f32 run 2x at best,
    and GPSIMD/ACT masks measured 10-20x off their modeled cost on HW),
  - 4 plain bf16 matmuls per chunk (lhsT = bf16 w column [128, 1])
    accumulate risk row 0 of four [128, 512] PSUM banks,
  - epilogue: risk -> clamp(3e-3) -> ln -> *e -> per-bank sums; plus
    prologue sum(theta*e) partials; host combines
    loss = -(sum(theta*e) - sum(e*ln r)) / N.
  numerics: bf16 rounds d_i (comparisons blur by ~2^-9 relative; measured
  rel err 1.8e-3 on the reference inputs vs the 2e-2 gate) and w (0.4%).
  The clamp guards rows whose self-term rounds out of its own risk set.

The jitted executable is built once and cached; a steady-state launch is a
single axon round-trip.
"""

import numpy as np

import concourse.bass as bass
import concourse.bacc as bacc
import concourse.mybir as mybir
from concourse.tile import TileContext

N = 16384
P = 128
NCH = N // P            # 128 chunks of j
NCORES = 8
NI = N // NCORES        # 2048 rows per core
FT = 512                # PSUM bank / i-tile width
NF = NI // FT           # 4 banks
NOUT = 2 * NF           # [0:4] sum(theta*e) quads, [4:8] sum(e*ln r) banks
IN_LEN = 2 * N + 3 * NI
F32 = mybir.dt.float32
F32R = mybir.dt.float32r
BF16 = mybir.dt.bfloat16
EXP = mybir.ActivationFunctionType.Exp
LN = mybir.ActivationFunctionType.Ln
COPY = mybir.ActivationFunctionType.Copy


def _build(mask_bufs=8):
    # disable_frame_to_traceback + the path scrub in _make_cached_runner keep
    # the BIR byte-identical across checkout paths, so the neuronx-cc compile
    # cache hits on a fresh copy of this file.
    nc = bacc.Bacc(num_devices=NCORES, disable_frame_to_traceback=True)
    packed = nc.declare_dram_parameter("packed", [IN_LEN], F32, isOutput=False)
    out = nc.declare_dram_parameter("partial", [NOUT, 1], F32, isOutput=True)

    with TileContext(nc) as tc:
        with (
            tc.tile_pool(name="const", bufs=1) as cpool,
            tc.tile_pool(name="mask", bufs=mask_bufs) as mpool,
            tc.tile_pool(name="acc", bufs=1, space="PSUM") as ppool,
            tc.tile_pool(name="bc", bufs=3, space="PSUM") as bcpool,
        ):
            sb_d = cpool.tile([P, NCH], F32)     # d[128p + c] at [p, c]
            sb_th = cpool.tile([P, NCH], F32)
            w_act = cpool.tile([P, NCH], F32)    # exp(theta)
            w16 = cpool.tile([P, NCH], BF16)     # bf16 stationary weights
            ones_row32 = cpool.tile([1, P], F32)
            ones_row = cpool.tile([1, P], F32R)  # bc lhsT [K=1, M=128]
            row_di = cpool.tile([1, NI], F32)
            row_di2 = cpool.tile([1, NI], F32R)
            row_ei = cpool.tile([1, NI], F32)
            thq = cpool.tile([NF, FT], F32)
            eq = cpool.tile([NF, FT], F32)
            bc_di16 = cpool.tile([P, NI], BF16)  # bf16 d_i broadcast
            clamp_row = cpool.tile([1, NI], F32)
            ln_row = cpool.tile([1, NI], F32)
            prod_row = cpool.tile([1, NI], F32)
            # one [1,1] tile per slot: multiple reduce writes into a shared
            # [1, NF] tile at free offsets silently land nowhere on HW
            slots = [cpool.tile([1, 1], F32, name=f"slot{t}") for t in range(NF)]
            prodq = cpool.tile([NF, FT], F32)
            partq = cpool.tile([NF, 1], F32)
            dummy = cpool.tile([1, 1], F32)
            one_c = cpool.tile([1, 1], F32)

            # ---- DMAs, spread across the DMA-capable queues; critical
            # chain: own-d row -> bc matmuls -> bc_di16 -> masks ----
            o_row = 2 * N
            nc.sync.dma_start(out=row_di[:, :],
                              in_=packed[o_row:o_row + NI].rearrange("(o n) -> o n", o=1))
            nc.sync.dma_start(out=sb_d[:, :],
                              in_=packed[:N].rearrange("(p c) -> p c", p=P))
            nc.gpsimd.dma_start(out=sb_th[:, :],
                                in_=packed[N:2 * N].rearrange("(p c) -> p c", p=P))
            nc.gpsimd.dma_start(
                out=thq[:, :],
                in_=packed[o_row + NI:o_row + 2 * NI].rearrange("(a b) -> a b", a=NF))
            nc.gpsimd.dma_start(
                out=eq[:, :],
                in_=packed[o_row + 2 * NI:o_row + 3 * NI].rearrange("(a b) -> a b", a=NF))
            nc.scalar.dma_start(
                out=row_ei[:, :],
                in_=packed[o_row + 2 * NI:o_row + 3 * NI].rearrange("(o n) -> o n", o=1))

            # ---- bc broadcast chain (f32r rhs: 1 cyc/row even cold) ----
            nc.vector.memset(ones_row32[:, :], 1.0)
            nc.vector.tensor_copy(ones_row[:, :], ones_row32[:, :])
            nc.vector.memset(one_c[:, :], 1.0)
            nc.vector.tensor_copy(row_di2[:, :], row_di[:, :])
            for t in range(NF):
                bc_ps = bcpool.tile([P, FT], F32, tag="bc")
                nc.tensor.matmul(
                    bc_ps[:, :], lhsT=ones_row[:, :],
                    rhs=row_di2[:, t * FT:(t + 1) * FT], start=True, stop=True,
                )
                # bf16 copies split DVE/ACT so neither serializes the chain
                if t % 2 == 0:
                    nc.vector.tensor_copy(bc_di16[:, t * FT:(t + 1) * FT],
                                          bc_ps[:, :])
                else:
                    nc.scalar.activation(bc_di16[:, t * FT:(t + 1) * FT],
                                         bc_ps[:, :], COPY)

            # per-half Exp + bf16 weight prep so early chunks unblock
            for h in range(2):
                hs = slice(h * (NCH // 2), (h + 1) * (NCH // 2))
                nc.scalar.activation(w_act[:, hs], sb_th[:, hs], EXP)
                nc.vector.tensor_copy(w16[:, hs], w_act[:, hs])

            # prologue half of the loss: sum(theta*e) quad partials
            nc.vector.tensor_mul(prodq[:, :], thq[:, :], eq[:, :])
            nc.vector.reduce_sum(partq[:, :], prodq[:, :],
                                 axis=mybir.AxisListType.X)
            nc.sync.dma_start(out=out[:NF, :], in_=partq[:, :])

            # ---- main loop: 128 chunks, one DVE 4x bf16 is_le mask each ----
            risk_ps = [ppool.tile([P, FT], F32, name=f"risk{t}") for t in range(NF)]
            for ci, c in enumerate(range(NCH)):
                m = mpool.tile([P, NI], BF16, tag="m", name=f"m{c}")
                nc.vector.tensor_scalar(
                    m[:, :], bc_di16[:, :], sb_d[:, c:c + 1], None,
                    mybir.AluOpType.is_le)
                for t in range(NF):
                    nc.tensor.matmul(
                        risk_ps[t][0:1, :], lhsT=w16[:, c:c + 1],
                        rhs=m[:, t * FT:(t + 1) * FT],
                        start=(ci == 0), stop=(ci == NCH - 1))

            # preload the Ln table while the mask stream still runs
            nc.scalar.activation(dummy[:, :], one_c[:, :], LN)

            # ---- tail: clamp, Ln from PSUM, mul by e, per-bank reduce ----
            for t in range(NF):
                sl = slice(t * FT, (t + 1) * FT)
                # bf16-rounded d_i can exclude a max-duration row from its
                # own risk set; the floor bounds that row's ln error
                nc.vector.tensor_scalar_max(clamp_row[:, sl],
                                            risk_ps[t][0:1, :], 3e-3)
                nc.scalar.activation(ln_row[:, sl], clamp_row[:, sl], LN)
                nc.vector.tensor_mul(prod_row[:, sl], ln_row[:, sl], row_ei[:, sl])
                nc.vector.reduce_sum(slots[t][:, :], prod_row[:, sl],
                                     axis=mybir.AxisListType.X)
                nc.sync.dma_start(out=out[NF + t:NF + t + 1, :],
                                  in_=slots[t][:, :])

    nc.finalize()
    return nc


def _make_cached_runner(nc):
    """One-time: lower nc to a jitted shard_map executable and keep it."""
    import jax
    from jax.experimental.shard_map import shard_map
    from jax.sharding import Mesh, PartitionSpec

    from concourse.bass2jax import (
        _bass_exec_p,
        install_neuronx_cc_hook,
        partition_id_tensor,
    )

    install_neuronx_cc_hook()

    # Scrub this file's absolute path out of the serialized BIR so the
    # compile-cache key is independent of the checkout directory.
    import os
    _path = os.path.abspath(__file__).encode()
    _orig_to_json_bytes = nc.to_json_bytes

    def _scrubbed_to_json_bytes():
        return _orig_to_json_bytes().replace(_path, b"kernel.py")

    nc.to_json_bytes = _scrubbed_to_json_bytes

    partition_name = nc.partition_id_tensor.name if nc.partition_id_tensor else None

    in_names, out_names, out_avals, out_shapes = [], [], [], []
    for alloc in nc.m.functions[0].allocations:
        if not isinstance(alloc, mybir.MemoryLocationSet):
            continue
        name = alloc.memorylocations[0].name
        if alloc.kind == "ExternalInput":
            if name != partition_name:
                in_names.append(name)
        elif alloc.kind == "ExternalOutput":
            out_names.append(name)
            shape = tuple(alloc.tensor_shape)
            out_shapes.append(shape)
            out_avals.append(jax.core.ShapedArray(shape, mybir.dt.np(alloc.dtype)))
    assert in_names == ["packed"] and out_names == ["partial"]
    n_params = len(in_names)
    n_outs = len(out_avals)
    all_in_names = in_names + out_names
    if partition_name is not None:
        all_in_names.append(partition_name)
    donate = tuple(range(n_params, n_params + n_outs))

    def _body(*args):
        operands = list(args)
        if partition_name is not None:
            operands.append(partition_id_tensor())
        outs = _bass_exec_p.bind(
            *operands,
            out_avals=tuple(out_avals),
            in_names=tuple(all_in_names),
            out_names=tuple(out_names),
            lowering_input_output_aliases=(),
            sim_require_finite=True,
            sim_require_nnan=True,
            nc=nc,
        )
        return tuple(outs)

    devices = jax.devices()[:NCORES]
    assert len(devices) == NCORES, f"need {NCORES} cores, have {len(jax.devices())}"
    mesh = Mesh(np.asarray(devices), ("core",))
    in_specs = (PartitionSpec("core"),) * (n_params + n_outs)
    out_specs = (PartitionSpec("core"),) * n_outs
    sharded = jax.jit(
        shard_map(_body, mesh=mesh, in_specs=in_specs, out_specs=out_specs,
                  check_rep=False),
        donate_argnums=donate,
        keep_unused=True,
    )

    def run(packed_all):
        """packed_all: [NCORES * IN_LEN] f32 -> per-core partials [NCORES, NOUT]."""
        zeros = np.zeros((NCORES * NOUT, 1), np.float32)
        out_arrs = sharded(packed_all, zeros)
        return np.asarray(out_arrs[0]).reshape(NCORES, NOUT)

    return run


_RUNNER = None


def _get_runner():
    global _RUNNER
    if _RUNNER is None:
        _RUNNER = _make_cached_runner(_build())
    return _RUNNER


def _pack_inputs(hazard_pred, durations, events):
    theta = np.asarray(hazard_pred, dtype=np.float32).reshape(-1)
    d = np.asarray(durations, dtype=np.float32).reshape(-1)
    e = np.asarray(events, dtype=np.float32).reshape(-1)
    pa = np.empty((NCORES, IN_LEN), np.float32)
    pa[:, :N] = d                      # replicated durations
    pa[:, N:2 * N] = theta             # replicated theta
    o = 2 * N
    pa[:, o:o + NI] = d.reshape(NCORES, NI)              # own rows' d
    pa[:, o + NI:o + 2 * NI] = theta.reshape(NCORES, NI)  # own rows' theta
    pa[:, o + 2 * NI:] = e.reshape(NCORES, NI)            # own rows' events
    return pa.reshape(-1)


def kernel(hazard_pred, durations, events):
    runner = _get_runner()
    partials = runner(_pack_inputs(hazard_pred, durations, events))
    per_core = (partials[:, :NF].astype(np.float64).sum(1)
                - partials[:, NF:].astype(np.float64).sum(1))
    loss = -(per_core.sum() / N)
    return np.asarray(loss, dtype=np.float32)


def run(hazard_pred, durations, events, **kw):
    """test.py compatibility shim."""
    return kernel(hazard_pred, durations, events), None


# revision 7
# speedup vs baseline: 363.9775x; 1.1587x over previous
"""Cox proportional-hazards loss (CoxNNet) on 8 Trainium2 NeuronCores.

loss = -mean((theta - log(risk_sum)) * events)
risk_sum[i] = sum_j [d_j >= d_i] * exp(theta_j)        (N = 16384)

Rows i of the [N, N] risk-set reduction are split across 8 cores (2048 rows
each); durations/theta are host-replicated to every core (the per-launch
wall is round-trip dominated, and skipping the on-device AllGather removes
~20us of collective latency from the kernel's critical path).

Per-core kernel (measured ~170us on HW vs ~1.1ms for the f32r/GPSIMD
baseline measured the same way):
  - d, theta land in a [128, 128] chunk layout (global j = 128p + c at
    [p, c]; per-partition-contiguous DMA),
  - bc_di = broadcast of the core's 2048 d_i values across partitions via
    K=1 f32r matmuls, copied out once in f32 and once in bf16,
  - per chunk c (128 j's): ONE DVE tensor_scalar is_le builds the
    [128, 2048] bf16 mask from the bf16 broadcast - bf16 in + bf16 out
    engages the DVE 4x perf mode (~0.5us/chunk; fp8 or f32 run 2x at best,
    and GPSIMD/ACT masks measured 10-20x off their modeled cost on HW),
  - 4 plain bf16 matmuls per chunk (lhsT = bf16 w column [128, 1])
    accumulate risk row 0 of four [128, 512] PSUM banks,
  - epilogue: risk -> clamp(3e-3) -> ln -> *e -> per-bank sums; plus
    prologue sum(theta*e) partials; host combines
    loss = -(sum(theta*e) - sum(e*ln r)) / N.
  numerics: bf16 rounds d_i (comparisons blur by ~2^-9 relative; measured
  rel err 1.8e-3 on the reference inputs vs the 2e-2 gate) and w (0.4%).
  The clamp guards rows whose self-term rounds out of its own risk set.

The jitted executable is built once and cached; a steady-state launch is a
single axon round-trip.
"""

import numpy as np

import concourse.bass as bass
import concourse.bacc as bacc
import concourse.mybir as mybir
from concourse.tile import TileContext

N = 16384
P = 128
NCH = N // P            # 128 chunks of j
NCORES = 8
NI = N // NCORES        # 2048 rows per core
FT = 512                # PSUM bank / i-tile width
NF = NI // FT           # 4 banks
NOUT = 2 * NF           # [0:4] sum(theta*e) quads, [4:8] sum(e*ln r) banks
IN_LEN = 2 * N + 3 * NI
F32 = mybir.dt.float32
F32R = mybir.dt.float32r
BF16 = mybir.dt.bfloat16
FP8 = mybir.dt.float8e4
DR = mybir.MatmulPerfMode.DoubleRow
N8 = 44                 # fp8 DoubleRow chunks (PE 0.43us/chunk, DVE 1.0)
                        # vs bf16 chunks (PE 0.85, DVE 0.55): 44/84 levels
                        # both engines at ~90us instead of PE-bound 109
EXP = mybir.ActivationFunctionType.Exp
LN = mybir.ActivationFunctionType.Ln
COPY = mybir.ActivationFunctionType.Copy


def _build(mask_bufs=8):
    # disable_frame_to_traceback + the path scrub in _make_cached_runner keep
    # the BIR byte-identical across checkout paths, so the neuronx-cc compile
    # cache hits on a fresh copy of this file.
    nc = bacc.Bacc(num_devices=NCORES, disable_frame_to_traceback=True)
    packed = nc.declare_dram_parameter("packed", [IN_LEN], F32, isOutput=False)
    out = nc.declare_dram_parameter("partial", [NOUT, 1], F32, isOutput=True)

    with TileContext(nc) as tc:
        with (
            tc.tile_pool(name="const", bufs=1) as cpool,
            tc.tile_pool(name="mask", bufs=mask_bufs) as mpool,
            tc.tile_pool(name="acc", bufs=1, space="PSUM") as ppool,
            tc.tile_pool(name="bc", bufs=3, space="PSUM") as bcpool,
        ):
            sb_d = cpool.tile([P, NCH], F32)     # d[128p + c] at [p, c]
            sb_th = cpool.tile([P, NCH], F32)
            w_act = cpool.tile([P, NCH], F32)    # exp(theta)
            w16 = cpool.tile([P, NCH], BF16)     # bf16 stationary weights
            # fp8 DoubleRow stationary: [p, q, c], q=0 w_hi / q=1 w_lo, cols
            # [NCH, 2*NCH) initialized pad for the sliding [c, c+128) window
            w8 = cpool.tile([P, 2, 2 * NCH], FP8)
            w8_up = cpool.tile([P, NCH], F32)
            w8_res = cpool.tile([P, NCH], F32)
            zeroq = cpool.tile([P, 4 * NCH], F32)
            ones_row32 = cpool.tile([1, P], F32)
            ones_row = cpool.tile([1, P], F32R)  # bc lhsT [K=1, M=128]
            row_di = cpool.tile([1, NI], F32)
            row_di2 = cpool.tile([1, NI], F32R)
            row_ei = cpool.tile([1, NI], F32)
            thq = cpool.tile([NF, FT], F32)
            eq = cpool.tile([NF, FT], F32)
            bc_di16 = cpool.tile([P, NI], BF16)  # bf16 d_i broadcast
            clamp_row = cpool.tile([1, NI], F32)
            ln_row = cpool.tile([1, NI], F32)
            prod_row = cpool.tile([1, NI], F32)
            # one [1,1] tile per slot: multiple reduce writes into a shared
            # [1, NF] tile at free offsets silently land nowhere on HW
            slots = [cpool.tile([1, 1], F32, name=f"slot{t}") for t in range(NF)]
            prodq = cpool.tile([NF, FT], F32)
            partq = cpool.tile([NF, 1], F32)
            dummy = cpool.tile([1, 1], F32)
            one_c = cpool.tile([1, 1], F32)

            # ---- DMAs, spread across the DMA-capable queues; critical
            # chain: own-d row -> bc matmuls -> bc_di16 -> masks ----
            o_row = 2 * N
            nc.sync.dma_start(out=row_di[:, :],
                              in_=packed[o_row:o_row + NI].rearrange("(o n) -> o n", o=1))
            nc.sync.dma_start(out=sb_d[:, :],
                              in_=packed[:N].rearrange("(p c) -> p c", p=P))
            nc.gpsimd.dma_start(out=sb_th[:, :],
                                in_=packed[N:2 * N].rearrange("(p c) -> p c", p=P))
            nc.gpsimd.dma_start(
                out=thq[:, :],
                in_=packed[o_row + NI:o_row + 2 * NI].rearrange("(a b) -> a b", a=NF))
            nc.gpsimd.dma_start(
                out=eq[:, :],
                in_=packed[o_row + 2 * NI:o_row + 3 * NI].rearrange("(a b) -> a b", a=NF))
            nc.scalar.dma_start(
                out=row_ei[:, :],
                in_=packed[o_row + 2 * NI:o_row + 3 * NI].rearrange("(o n) -> o n", o=1))

            # ---- bc broadcast chain (f32r rhs: 1 cyc/row even cold) ----
            nc.vector.memset(ones_row32[:, :], 1.0)
            nc.vector.tensor_copy(ones_row[:, :], ones_row32[:, :])
            nc.vector.memset(one_c[:, :], 1.0)
            # fp8 memset is invalid ISA; zero the w8 pad via an f32 copy
            nc.vector.memset(zeroq[:, :], 0.0)
            nc.vector.tensor_copy(w8[:, :, :].rearrange("p a c -> p (a c)"),
                                  zeroq[:, :])
            nc.vector.tensor_copy(row_di2[:, :], row_di[:, :])
            for t in range(NF):
                bc_ps = bcpool.tile([P, FT], F32, tag="bc")
                nc.tensor.matmul(
                    bc_ps[:, :], lhsT=ones_row[:, :],
                    rhs=row_di2[:, t * FT:(t + 1) * FT], start=True, stop=True,
                )
                # bf16 copies split DVE/ACT so neither serializes the chain
                if t % 2 == 0:
                    nc.vector.tensor_copy(bc_di16[:, t * FT:(t + 1) * FT],
                                          bc_ps[:, :])
                else:
                    nc.scalar.activation(bc_di16[:, t * FT:(t + 1) * FT],
                                         bc_ps[:, :], COPY)

            # per-half Exp + bf16 weight prep so early chunks unblock
            for h in range(2):
                hs = slice(h * (NCH // 2), (h + 1) * (NCH // 2))
                nc.scalar.activation(w_act[:, hs], sb_th[:, hs], EXP)
                nc.vector.tensor_copy(w16[:, hs], w_act[:, hs])
                nc.vector.tensor_copy(w8[:, 0, hs], w_act[:, hs])
                nc.vector.tensor_copy(w8_up[:, hs], w8[:, 0, hs])
                nc.vector.tensor_sub(w8_res[:, hs], w_act[:, hs], w8_up[:, hs])
                nc.vector.tensor_copy(w8[:, 1, hs], w8_res[:, hs])

            # prologue half of the loss: sum(theta*e) quad partials
            nc.vector.tensor_mul(prodq[:, :], thq[:, :], eq[:, :])
            nc.vector.reduce_sum(partq[:, :], prodq[:, :],
                                 axis=mybir.AxisListType.X)
            nc.sync.dma_start(out=out[:NF, :], in_=partq[:, :])

            # ---- main loop: 128 chunks, one DVE 4x bf16 is_le mask each ----
            risk_ps = [ppool.tile([P, FT], F32, name=f"risk{t}") for t in range(NF)]
            # spread the fp8 chunks evenly; chunk 0 must be fp8 so its
            # start=True zeroes the full [128, FT] bank (bf16 matmuls only
            # touch row 0; DoubleRow's unread rows 1..127 stay defined)
            is8 = [((c * N8) // NCH) != (((c - 1) * N8) // NCH) or c == 0
                   for c in range(NCH)]
            for ci, c in enumerate(range(NCH)):
                if is8[c]:
                    m8 = mpool.tile([P, NI], FP8, tag="m8", name=f"m8{c}")
                    nc.vector.tensor_scalar(
                        m8[:, :], bc_di16[:, :], sb_d[:, c:c + 1], None,
                        mybir.AluOpType.is_le)
                    lhsT = w8[:, :, c:c + P]
                    for t in range(NF):
                        rhs = m8[:, t * FT:(t + 1) * FT].rearrange(
                            "p (a f) -> p a f", a=1).broadcast_to([P, 2, FT])
                        nc.tensor.matmul(
                            risk_ps[t][:, :], lhsT=lhsT, rhs=rhs,
                            start=(ci == 0), stop=(ci == NCH - 1),
                            perf_mode=DR)
                    continue
                m = mpool.tile([P, NI], BF16, tag="m", name=f"m{c}")
                nc.vector.tensor_scalar(
                    m[:, :], bc_di16[:, :], sb_d[:, c:c + 1], None,
                    mybir.AluOpType.is_le)
                for t in range(NF):
                    nc.tensor.matmul(
                        risk_ps[t][0:1, :], lhsT=w16[:, c:c + 1],
                        rhs=m[:, t * FT:(t + 1) * FT],
                        start=(ci == 0), stop=(ci == NCH - 1))

            # preload the Ln table while the mask stream still runs
            nc.scalar.activation(dummy[:, :], one_c[:, :], LN)

            # ---- tail: clamp, Ln from PSUM, mul by e, per-bank reduce ----
            for t in range(NF):
                sl = slice(t * FT, (t + 1) * FT)
                # bf16-rounded d_i can exclude a max-duration row from its
                # own risk set; the floor bounds that row's ln error
                nc.vector.tensor_scalar_max(clamp_row[:, sl],
                                            risk_ps[t][0:1, :], 3e-3)
                nc.scalar.activation(ln_row[:, sl], clamp_row[:, sl], LN)
                nc.vector.tensor_mul(prod_row[:, sl], ln_row[:, sl], row_ei[:, sl])
                nc.vector.reduce_sum(slots[t][:, :], prod_row[:, sl],
                                     axis=mybir.AxisListType.X)
                nc.sync.dma_start(out=out[NF + t:NF + t + 1, :],
                                  in_=slots[t][:, :])

    nc.finalize()
    return nc


def _make_cached_runner(nc):
    """One-time: lower nc to a jitted shard_map executable and keep it."""
    import jax
    from jax.experimental.shard_map import shard_map
    from jax.sharding import Mesh, PartitionSpec

    from concourse.bass2jax import (
        _bass_exec_p,
        install_neuronx_cc_hook,
        partition_id_tensor,
    )

    install_neuronx_cc_hook()

    # Scrub this file's absolute path out of the serialized BIR so the
    # compile-cache key is independent of the checkout directory.
    import os
    _path = os.path.abspath(__file__).encode()
    _orig_to_json_bytes = nc.to_json_bytes

    def _scrubbed_to_json_bytes():
        return _orig_to_json_bytes().replace(_path, b"kernel.py")

    nc.to_json_bytes = _scrubbed_to_json_bytes

    partition_name = nc.partition_id_tensor.name if nc.partition_id_tensor else None

    in_names, out_names, out_avals, out_shapes = [], [], [], []
    for alloc in nc.m.functions[0].allocations:
        if not isinstance(alloc, mybir.MemoryLocationSet):
            continue
        name = alloc.memorylocations[0].name
        if alloc.kind == "ExternalInput":
            if name != partition_name:
                in_names.append(name)
        elif alloc.kind == "ExternalOutput":
            out_names.append(name)
            shape = tuple(alloc.tensor_shape)
            out_shapes.append(shape)
            out_avals.append(jax.core.ShapedArray(shape, mybir.dt.np(alloc.dtype)))
    assert in_names == ["packed"] and out_names == ["partial"]
    n_params = len(in_names)
    n_outs = len(out_avals)
    all_in_names = in_names + out_names
    if partition_name is not None:
        all_in_names.append(partition_name)
    donate = tuple(range(n_params, n_params + n_outs))

    def _body(*args):
        operands = list(args)
        if partition_name is not None:
            operands.append(partition_id_tensor())
        outs = _bass_exec_p.bind(
            *operands,
            out_avals=tuple(out_avals),
            in_names=tuple(all_in_names),
            out_names=tuple(out_names),
            lowering_input_output_aliases=(),
            sim_require_finite=True,
            sim_require_nnan=True,
            nc=nc,
        )
        return tuple(outs)

    devices = jax.devices()[:NCORES]
    assert len(devices) == NCORES, f"need {NCORES} cores, have {len(jax.devices())}"
    mesh = Mesh(np.asarray(devices), ("core",))
    in_specs = (PartitionSpec("core"),) * (n_params + n_outs)
    out_specs = (PartitionSpec("core"),) * n_outs
    sharded = jax.jit(
        shard_map(_body, mesh=mesh, in_specs=in_specs, out_specs=out_specs,
                  check_rep=False),
        donate_argnums=donate,
        keep_unused=True,
    )

    def run(packed_all):
        """packed_all: [NCORES * IN_LEN] f32 -> per-core partials [NCORES, NOUT]."""
        zeros = np.zeros((NCORES * NOUT, 1), np.float32)
        out_arrs = sharded(packed_all, zeros)
        return np.asarray(out_arrs[0]).reshape(NCORES, NOUT)

    return run


_RUNNER = None


def _get_runner():
    global _RUNNER
    if _RUNNER is None:
        _RUNNER = _make_cached_runner(_build())
    return _RUNNER


def _pack_inputs(hazard_pred, durations, events):
    theta = np.asarray(hazard_pred, dtype=np.float32).reshape(-1)
    d = np.asarray(durations, dtype=np.float32).reshape(-1)
    e = np.asarray(events, dtype=np.float32).reshape(-1)
    pa = np.empty((NCORES, IN_LEN), np.float32)
    pa[:, :N] = d                      # replicated durations
    pa[:, N:2 * N] = theta             # replicated theta
    o = 2 * N
    pa[:, o:o + NI] = d.reshape(NCORES, NI)              # own rows' d
    pa[:, o + NI:o + 2 * NI] = theta.reshape(NCORES, NI)  # own rows' theta
    pa[:, o + 2 * NI:] = e.reshape(NCORES, NI)            # own rows' events
    return pa.reshape(-1)


def kernel(hazard_pred, durations, events):
    runner = _get_runner()
    partials = runner(_pack_inputs(hazard_pred, durations, events))
    per_core = (partials[:, :NF].astype(np.float64).sum(1)
                - partials[:, NF:].astype(np.float64).sum(1))
    loss = -(per_core.sum() / N)
    return np.asarray(loss, dtype=np.float32)


def run(hazard_pred, durations, events, **kw):
    """test.py compatibility shim."""
    return kernel(hazard_pred, durations, events), None
